# revision 48
# baseline (speedup 1.0000x reference)
"""GPT forward pass (B=2,T=1024,C=768,H=12,L=6,V=32000) on 8 TRN2 NeuronCores.

Sharding: context/token parallel. Token blocks of 128; batch bt=r//4, local
rank lr=r%4; core r owns query blocks {lr, 7-lr} of its batch (balanced causal
work). Per layer the LN1 activations x-hat (bf16) are all-gathered within each
4-core batch group as TWO half-column collectives (first the early global
blocks 0..3, then 4..7). K/V for the full sequence are recomputed locally from
the gathered x-hat (PE matmul cost is output-columns only, so recompute beats
shipping K/V). LM head is token-parallel (each core: own 256 tokens x full
vocab) so no final collective is needed.

Software pipelining: the per-layer tail (proj/LN2/FFN/LN1') is split per
owned token block so that block A's next-layer LN1 completes mid-layer and
launches gather-half-0 of layer l+1 while block B is still computing; KV0 /
attention / FFN work of layer l then overlaps both in-flight collectives.
Steady state hides nearly all of the ~35us-per-collective latency.

Attention computes scores transposed, S^T[k,q] = K Q^T, over a rank-uniform
fixed set of kv blocks (4 for the early query block, 8 for the late one);
causality and the rank-varying diagonal live in a host-supplied 0/1 mask
multiplied into exp(S^T). No row-max is needed (scores are O(0.1)); the
softmax denominator falls out of a ones-column appended to V, and
normalization is a rank-1 broadcast matmul. No transposes anywhere.

Activations stay feature-major [C_part, token_free]; LN stats/broadcasts via
rank-1 bf16 PE matmuls.
"""

import sys

for _p in (
    "/opt/trn_rl_repo",
    "/opt/pypackages",
    "/root/.axon_site",
    "/root/.axon_site/_ro/trn_rl_repo",
    "/root/.axon_site/_ro/pypackages",
):
    if _p not in sys.path:
        sys.path.append(_p)

import numpy as np
import ml_dtypes

import concourse.bass as bass
import concourse.mybir as mybir
import concourse.tile as tile
from concourse import bacc
from concourse.bass_utils import run_bass_kernel_spmd

BF16 = mybir.dt.bfloat16
F32 = mybir.dt.float32
AF = mybir.ActivationFunctionType
OP = mybir.AluOpType

B, T, C, H, L, V = 2, 1024, 768, 12, 6, 32000
HS, P = 64, 128
NCORES = 8
FT = C // P  # 6 feature tiles
F4 = 4 * C // P  # 24 ffn tiles
TB = 256  # tokens per core
NB = T // P  # 8 blocks per batch sequence
VC2 = 1024  # vocab chunk for the head
EPS = 1e-5
SCALE = C ** -0.5
QW = (4, 8)  # rank-uniform kv-block widths for the two owned query blocks
# global block gb -> column offset in shard-ordered full-sequence buffers
# (shard s contributes its blocks s and 7-s at column s*TB and s*TB+P)
COL = [0, 256, 512, 768, 896, 640, 384, 128]
SHX2 = FT * P * P  # 98304 bf16 elements per half-shard


def _blocks_of(rank):
    lr = rank % 4
    return [lr, 7 - lr]


def build(n_layers=L, attn=True, head=True):
    nc = bacc.Bacc("TRN2", target_bir_lowering=False, debug=False,
                   num_devices=NCORES)

    x0_d = nc.dram_tensor("x0", [P, FT, TB], BF16, kind="ExternalInput")
    wqkv_d = nc.dram_tensor("wqkv", [L, FT, P, 3 * C], BF16, kind="ExternalInput")
    wp_d = nc.dram_tensor("wp", [L, FT, P, C], BF16, kind="ExternalInput")
    w1_d = nc.dram_tensor("w1", [L, FT, P, 4 * C], BF16, kind="ExternalInput")
    w2_d = nc.dram_tensor("w2", [L, F4, P, C], BF16, kind="ExternalInput")
    wh_d = nc.dram_tensor("wh", [FT, P, V], BF16, kind="ExternalInput")
    lng_d = nc.dram_tensor("lng", [2 * L + 1, C], BF16, kind="ExternalInput")
    lnb_d = nc.dram_tensor("lnb", [2 * L + 1, C], F32, kind="ExternalInput")
    bp_d = nc.dram_tensor("bp", [L, C], F32, kind="ExternalInput")
    b1_d = nc.dram_tensor("b1", [L, 4 * C], F32, kind="ExternalInput")
    b2_d = nc.dram_tensor("b2", [L, C], F32, kind="ExternalInput")
    msk_d = nc.dram_tensor("msk", [P, 8 * P], BF16, kind="ExternalInput")
    out_d = nc.dram_tensor("out", [2, P, V], BF16, kind="ExternalOutput")

    with tile.TileContext(nc) as tc:
        with (
            tc.tile_pool(name="const", bufs=1) as cp,
            tc.tile_pool(name="act", bufs=1) as ap,
            tc.tile_pool(name="rows", bufs=2) as rp,
            tc.tile_pool(name="dram", bufs=1, space="DRAM") as dp,
        ):
            # ---- constants ----
            ones_col_bf = cp.tile([P, 1], BF16, name="ones_col_bf")
            nc.vector.memset(ones_col_bf[:], 1.0)
            ones_row_bf = cp.tile([1, P], BF16, name="ones_row_bf")
            nc.vector.memset(ones_row_bf[:], 1.0)
            eps_c = cp.tile([1, 1], F32, name="eps_c")
            nc.vector.memset(eps_c[:], EPS)
            msk = cp.tile([P, 8 * P], BF16, name="msk")
            nc.scalar.dma_start(msk[:], msk_d.ap())

            with tc.tile_pool(name="psum", bufs=1, space="PSUM") as pp:

                def ln_params(i):
                    g_row = rp.tile([1, C], BF16, tag="grow")
                    nc.scalar.dma_start(g_row[:], lng_d.ap()[i : i + 1, :])
                    b_f32 = rp.tile([1, C], F32, tag="bf32")
                    nc.scalar.dma_start(b_f32[:], lnb_d.ap()[i : i + 1, :])
                    b_row = rp.tile([1, C], BF16, tag="brow")
                    nc.vector.tensor_copy(b_row[:], b_f32[:])
                    return g_row, b_row

                def ln_block(params, x_src, c0, out, w=P, oc0=None, mid=None):
                    """LN of x_src[:, :, c0:c0+w] -> out[:, :, oc0:oc0+w].
                    Apply processes f-tiles in pairs (one DVE op covers two)
                    with the bias folded into the ga2 broadcast by PE.
                    `mid` emits independent PE work between the stats chain
                    and the apply so the chain's DVE latency is hidden."""
                    if oc0 is None:
                        oc0 = c0
                    g_row, b_row = params
                    st1 = pp.tile([1, TB], F32, tag="o", bufs=3)
                    st2 = pp.tile([1, TB], F32, tag="o", bufs=3)
                    for f in range(FT):
                        sq = ap.tile([P, w], BF16, tag="sq", bufs=2)
                        # squares on the otherwise-idle GpSimd engine to keep
                        # the DVE queue clear for psum-releasing ops
                        nc.gpsimd.tensor_tensor(sq[:],
                                                x_src[:, f, c0 : c0 + w],
                                                x_src[:, f, c0 : c0 + w],
                                                op=OP.mult)
                        nc.tensor.matmul(st1[:1, 0:w], ones_col_bf[:],
                                         x_src[:, f, c0 : c0 + w],
                                         start=(f == 0), stop=(f == FT - 1))
                        nc.tensor.matmul(st2[:1, 0:w], ones_col_bf[:], sq[:],
                                         start=(f == 0), stop=(f == FT - 1))
                    rinv, mean, m2, var = (
                        rp.tile([1, w], F32, tag=t, name=t)[:1, :]
                        for t in ("rinv", "mean", "m2", "var")
                    )
                    nc.vector.tensor_scalar_mul(mean, st1[:1, 0:w], 1.0 / C)
                    nc.vector.tensor_tensor(m2, mean, mean, op=OP.mult)
                    nc.vector.scalar_tensor_tensor(
                        var, in0=st2[:1, 0:w], scalar=1.0 / C, in1=m2,
                        op0=OP.mult, op1=OP.subtract,
                    )
                    std = rp.tile([1, w], F32, tag="std", name="std")[:1, :]
                    nc.scalar.activation(std, var, AF.Sqrt, bias=eps_c[:1, :1])
                    rinv_b = rp.tile([1, w], BF16, tag="rinvb", name="rinvb")
                    nmr_b = rp.tile([1, w], BF16, tag="nmrb", name="nmrb")
                    with nc.allow_low_precision(reason="ln scale rows bf16, "
                                                "same as the former f32+copy"):
                        nc.vector.reciprocal(rinv, std)
                        nc.vector.tensor_copy(rinv_b[:1, :], rinv)
                        nc.vector.scalar_tensor_tensor(
                            nmr_b[:1, :], in0=mean, scalar=-1.0, in1=rinv,
                            op0=OP.mult, op1=OP.mult,
                        )
                    if mid is not None:
                        mid()
                    for fp in range(FT // 2):
                        f0 = 2 * fp
                        ga = pp.tile([P, 4 * P], F32, tag="g", bufs=3)
                        for j in range(2):
                            fj = f0 + j
                            nc.tensor.matmul(
                                ga[:, j * w : j * w + w],
                                g_row[:1, fj * P : (fj + 1) * P],
                                rinv_b[:1, :], start=True, stop=True)
                            nc.tensor.matmul(
                                ga[:, (2 + j) * w : (2 + j) * w + w],
                                g_row[:1, fj * P : (fj + 1) * P],
                                nmr_b[:1, :], start=True, stop=False)
                            nc.tensor.matmul(
                                ga[:, (2 + j) * w : (2 + j) * w + w],
                                b_row[:1, fj * P : (fj + 1) * P],
                                ones_row_bf[:1, 0:w], start=False, stop=True)
                        t1 = ap.tile([P, 2, w], BF16, tag="t1", bufs=2)
                        nc.vector.tensor_tensor(
                            t1[:], x_src[:, f0 : f0 + 2, c0 : c0 + w],
                            ga[:, 0 : 2 * w], op=OP.mult)
                        nc.vector.tensor_tensor(
                            out[:, f0 : f0 + 2, oc0 : oc0 + w], t1[:],
                            ga[:, 2 * w : 4 * w], op=OP.add)

                # ---- embedding: host supplies feature-major bf16 tok+pos ----
                x_cur = ap.tile([P, FT, TB], BF16, tag="x", bufs=2)
                nc.sync.dma_start(x_cur[:], x0_d.ap())

                # ---- internal DRAM for the split x-hat all-gather ----
                ag_in = [dp.tile([1, SHX2], BF16, name=f"ag_in{i}")
                         for i in range(2)]
                ag_out = [dp.tile([4, SHX2], BF16, name=f"ag_out{i}")
                          for i in range(2)]

                def launch_gather(xh_t, hf):
                    """Stage xh block hf (local cols hf*P..) and all-gather.
                    Staged partition-major so the xf reload gets >=512B
                    contiguous elements (no 2x DMA latency penalty)."""
                    nc.sync.dma_start(
                        ag_in[hf][0].rearrange("(p f t) -> p f t", f=FT, t=P),
                        xh_t[:, :, hf * P : (hf + 1) * P],
                    )
                    nc.gpsimd.collective_compute(
                        "AllGather", OP.bypass,
                        replica_groups=[[0, 1, 2, 3], [4, 5, 6, 7]],
                        ins=[ag_in[hf][:].opt()],
                        outs=[ag_out[hf][:].opt()],
                    )

                # persistent V (token-major; 65th column stays 1.0 so the
                # softmax denominator falls out of the PV matmul)
                v_sb = ap.tile([P, NB, H, HS + 1], BF16, tag="vsb", bufs=1)
                nc.vector.memset(v_sb[:], 1.0)

                def alloc_xf():
                    # xf laid out [p, half, shard, f, t] so each shard load is
                    # one DMA with 1536B contiguous elements
                    return ap.tile([P, 2, 4, FT, P], BF16, tag="xf",
                                   name="xf")

                def alloc_kf():
                    return ap.tile([P, FT, T], BF16, tag="kf", name="kf")

                def kv_k(wq_t, xf_t, kf_t, hf):
                    """K for the 4 global blocks of half hf. One PSUM tile
                    per out-tile spanning all 4 shards -> one copy each;
                    copies alternate DVE/Act to halve queue pressure."""
                    for o in range(FT):
                        ps = pp.tile([P, 4 * P], F32, tag="g", bufs=3)
                        for s in range(4):
                            for f in range(FT):
                                nc.tensor.matmul(
                                    ps[:, s * P : (s + 1) * P],
                                    wq_t[:, f, C + o * P : C + (o + 1) * P],
                                    xf_t[:, hf, s, f, :],
                                    start=(f == 0), stop=(f == FT - 1),
                                )
                        # shard s of half hf -> kf cols s*TB + hf*P
                        nc.vector.tensor_copy(
                            kf_t[:, o, :].rearrange(
                                "p (s u r) -> p s u r", u=2, r=P)[:, :, hf, :],
                            ps[:].rearrange("p (s r) -> p s r", r=P))

                def kv_v(wq_t, xf_t, hf):
                    """V for the 4 global blocks of half hf, into v_sb.
                    Two 1-bank PSUM chunks per shard (heads 0-7 / 8-11)."""
                    for s in range(4):
                        vb = (s * TB + hf * P) // P
                        for i, (v0, v1, h0, h1) in enumerate(
                                ((0, 512, 0, 8), (512, C, 8, H))):
                            ps = pp.tile([P, 512], F32, tag="s", bufs=2)
                            for f in range(FT):
                                nc.tensor.matmul(
                                    ps[:, 0 : v1 - v0],
                                    xf_t[:, hf, s, f, :],
                                    wq_t[:, f, 2 * C + v0 : 2 * C + v1],
                                    start=(f == 0), stop=(f == FT - 1),
                                )
                            nc.vector.tensor_copy(
                                v_sb[:, vb, h0:h1, 0:HS],
                                ps[:, 0 : v1 - v0].rearrange(
                                    "p (h s) -> p h s", s=HS))

                def emit_xf_loads(xf_t, hf):
                    """SP-queue loads of gathered x-hat; emitted right after
                    the corresponding gather launch so they sit ahead of the
                    next staging DMA in the in-order SP queue."""
                    for s in range(4):
                        nc.sync.dma_start(
                            xf_t[:, hf, s, :, :],
                            ag_out[hf][s].rearrange(
                                "(p f t) -> p f t", f=FT, t=P),
                        )

                with tc.tile_pool(name="wts", bufs=1) as wp_pool:

                    def load_wqkv(l):
                        # bufs=1: the reload window (KV1(l) done -> Q(l+1)) is
                        # ~35us vs a ~10us DMA, so single-buffering is free
                        wq_t = wp_pool.tile([P, FT, 3 * C], BF16,
                                            tag="wqkv", bufs=1)
                        for j in range(3):
                            nc.scalar.dma_start(
                                wq_t[:, :, j * C : (j + 1) * C],
                                wqkv_d.ap()[l, :, :, j * C : (j + 1) * C]
                                .rearrange("f p m -> p f m"))
                        return wq_t

                    # ---- prelude: LN1(layer 0) + both gathers ----
                    wqkv_t = load_wqkv(0)
                    xh = ap.tile([P, FT, TB], BF16, tag="xh", bufs=2)
                    p_ln1 = ln_params(0)
                    ln_block(p_ln1, x_cur, 0, xh)
                    launch_gather(xh, 0)
                    ln_block(p_ln1, x_cur, P, xh)
                    launch_gather(xh, 1)
                    xf_cur = alloc_xf()
                    emit_xf_loads(xf_cur, 0)
                    emit_xf_loads(xf_cur, 1)

                    for l in range(n_layers):
                        # -- bias rows for this layer (Act queue) --
                        bpc = rp.tile([P, FT], F32, tag="bpc")
                        nc.scalar.dma_start(
                            bpc[:], bp_d.ap()[l].rearrange("(f p) -> p f", p=P))
                        b1c = rp.tile([P, F4], F32, tag="b1c")
                        nc.scalar.dma_start(
                            b1c[:], b1_d.ap()[l].rearrange("(f p) -> p f", p=P))
                        b2c = rp.tile([P, FT], F32, tag="b2c")
                        nc.scalar.dma_start(
                            b2c[:], b2_d.ap()[l].rearrange("(f p) -> p f", p=P))

                        # -- Q for own tokens (overlaps in-flight gathers) --
                        q_sb = ap.tile([P, FT, TB], BF16, tag="q")
                        for o in range(FT):
                            ps = pp.tile([P, TB], F32, tag="g", bufs=3)
                            for f in range(FT):
                                nc.tensor.matmul(
                                    ps[:], wqkv_t[:, f, o * P : (o + 1) * P],
                                    xh[:, f, :], start=(f == 0),
                                    stop=(f == FT - 1),
                                )
                            nc.scalar.copy(q_sb[:, o, :], ps[:])

                        # -- this layer's remaining weights (Act HWDGE) --
                        wp_t = wp_pool.tile([P, FT, C], BF16, tag="wp")
                        nc.scalar.dma_start(
                            wp_t[:], wp_d.ap()[l].rearrange("f p m -> p f m"))
                        w1c = []
                        for ch in range(2):
                            w1ct = wp_pool.tile([P, FT, 2 * C], BF16, tag="w1",
                                                bufs=2, name=f"w1c{ch}")
                            w1c.append(w1ct)
                            for j in range(2):
                                m0 = ch * 2 * C + j * C
                                nc.scalar.dma_start(
                                    w1ct[:, :, j * C : (j + 1) * C],
                                    w1_d.ap()[l, :, :, m0 : m0 + C]
                                    .rearrange("f p m -> p f m"))
                        w2c = []
                        for ch in range(2):
                            w2ct = wp_pool.tile([P, F4, 3 * P], BF16,
                                                tag="w2", bufs=2,
                                                name=f"w2c{ch}")
                            w2c.append(w2ct)
                            for j in range(2):
                                nc.scalar.dma_start(
                                    w2ct[:, j * 12 : (j + 1) * 12, :],
                                    w2_d.ap()[l, j * 12 : (j + 1) * 12, :,
                                              ch * 3 * P : (ch + 1) * 3 * P]
                                    .rearrange("f p m -> p f m"))

                        kf = alloc_kf()
                        xf = xf_cur
                        o_fm = ap.tile([P, FT, TB], BF16, tag="ofm")

                        def attn_block(ql):
                            """Attention for owned query block ql (0=early).
                            3-stage software pipeline over heads so PE always
                            has the next head's scores while the softmax
                            chains (exp/mask, recip) of earlier heads drain.
                            Scores/softmax chunked by 4 kv blocks so score
                            PSUM tiles stay one bank."""
                            W = QW[ql]

                            def stage_scores(h):
                                hp, f = HS * (h % 2), h // 2
                                pts = []
                                for c in range(W // 4):
                                    s_ps = pp.tile([P, 4 * P], F32, tag="s",
                                                   bufs=2)
                                    for j in range(4):
                                        kb = 4 * c + j
                                        nc.tensor.matmul(
                                            s_ps[:, j * P : (j + 1) * P],
                                            kf[hp : hp + HS, f,
                                               COL[kb] : COL[kb] + P],
                                            q_sb[hp : hp + HS, f,
                                                 ql * P : (ql + 1) * P],
                                            start=True, stop=True,
                                        )
                                    pt = ap.tile([P, 4 * P], BF16,
                                                 tag="pt", bufs=4, name="pt")
                                    nc.scalar.activation(pt[:], s_ps[:],
                                                         AF.Exp)
                                    # kv blocks 0..3 under a late query block
                                    # (>=4) are fully causal: no mask needed
                                    if ql == 0 or c == 1:
                                        nc.vector.tensor_tensor(
                                            pt[:], pt[:],
                                            msk[:, (0 if ql == 0 else 4 * P):]
                                            [:, : 4 * P], op=OP.mult)
                                    pts.append(pt)
                                return pts

                            def stage_av(h, pts):
                                ov = pp.tile([P, TB], F32, tag="o", bufs=3)
                                for c, pt in enumerate(pts):
                                    for j in range(4):
                                        kb = 4 * c + j
                                        nc.tensor.matmul(
                                            ov[: HS + 1, 0:P],
                                            v_sb[:, COL[kb] // P, h, :],
                                            pt[:, j * P : (j + 1) * P],
                                            start=(kb == 0),
                                            stop=(kb == W - 1),
                                        )
                                rden = rp.tile([1, P], BF16, tag="rden",
                                               bufs=2)
                                with nc.allow_low_precision(
                                        reason="softmax rden bf16"):
                                    nc.vector.reciprocal(rden[:1, :],
                                                         ov[HS : HS + 1, 0:P])
                                return ov, rden

                            def stage_fin(h, ov, rden):
                                hp, f = HS * (h % 2), h // 2
                                nc.tensor.matmul(
                                    ov[0:HS, P : P + P],
                                    ones_row_bf[:1, 0:HS], rden[:1, :],
                                    start=True, stop=True,
                                )
                                sc_sb = ap.tile([HS, P], BF16, tag="scsb",
                                                bufs=3)
                                nc.scalar.copy(sc_sb[:], ov[0:HS, P : P + P])
                                nc.vector.tensor_tensor(
                                    o_fm[hp : hp + HS, f,
                                         ql * P : (ql + 1) * P],
                                    ov[0:HS, 0:P], sc_sb[:],
                                    op=OP.mult,
                                )

                            pts = [None] * H
                            avs = [None] * H
                            for h in range(H):
                                pts[h] = stage_scores(h)
                                if h >= 1:
                                    avs[h - 1] = stage_av(h - 1, pts[h - 1])
                                if h >= 2:
                                    stage_fin(h - 2, *avs[h - 2])
                            avs[H - 1] = stage_av(H - 1, pts[H - 1])
                            stage_fin(H - 2, *avs[H - 2])
                            stage_fin(H - 1, *avs[H - 1])

                        def proj_block(c0, x_new):
                            """Output projection + residual for token block."""
                            for o in range(FT):
                                ps = pp.tile([P, TB], F32, tag="g", bufs=3)
                                for f in range(FT):
                                    nc.tensor.matmul(
                                        ps[:, 0:P],
                                        wp_t[:, f, o * P : (o + 1) * P],
                                        o_fm[:, f, c0 : c0 + P],
                                        start=(f == 0), stop=(f == FT - 1),
                                    )
                                nc.vector.scalar_tensor_tensor(
                                    x_new[:, o, c0 : c0 + P], in0=ps[:, 0:P],
                                    scalar=bpc[:, o : o + 1],
                                    in1=x_cur[:, o, c0 : c0 + P],
                                    op0=OP.add, op1=OP.add,
                                )

                        def ffn_block(c0, x_mid, xh2, x_fin):
                            """FFN + residual for token block at c0. xh2 is
                            block-local [P, FT, P]."""
                            h1 = ap.tile([P, F4, P], BF16, tag="h1", bufs=1)
                            for o in range(F4):
                                ps = pp.tile([P, TB], F32, tag="g", bufs=3)
                                for f in range(FT):
                                    nc.tensor.matmul(
                                        ps[:, 0:P],
                                        w1c[o // 12][:, f,
                                                     (o % 12) * P : (o % 12 + 1) * P],
                                        xh2[:, f, :],
                                        start=(f == 0), stop=(f == FT - 1),
                                    )
                                nc.scalar.activation(h1[:, o, :], ps[:, 0:P],
                                                     AF.Relu,
                                                     bias=b1c[:, o : o + 1])
                            for o in range(FT):
                                ps = pp.tile([P, TB], F32, tag="g", bufs=3)
                                for f in range(F4):
                                    nc.tensor.matmul(
                                        ps[:, 0:P],
                                        w2c[o // 3][:, f,
                                                    (o % 3) * P : (o % 3 + 1) * P],
                                        h1[:, f, :],
                                        start=(f == 0), stop=(f == F4 - 1),
                                    )
                                nc.vector.scalar_tensor_tensor(
                                    x_fin[:, o, c0 : c0 + P], in0=ps[:, 0:P],
                                    scalar=b2c[:, o : o + 1],
                                    in1=x_mid[:, o, c0 : c0 + P],
                                    op0=OP.add, op1=OP.add,
                                )

                        last = l == n_layers - 1

                        x_mid = ap.tile([P, FT, TB], BF16, tag="xm", bufs=2)
                        x_fin = ap.tile([P, FT, TB], BF16, tag="x", bufs=2)
                        xh_n = ap.tile([P, FT, TB], BF16, tag="xh", bufs=2)
                        p_ln2 = ln_params(2 * l + 1)

                        if attn:
                            # scheduler-only fence: keep the collective-gated
                            # KV work from being hoisted ahead of ready work
                            tc.no_sync_barrier()
                            kv_k(wqkv_t, xf, kf, 0)
                            kv_v(wqkv_t, xf, 0)
                            attn_block(0)
                        else:
                            for f in range(FT):
                                nc.vector.tensor_copy(o_fm[:, f, :],
                                                      xh[:, f, :])

                        # ---- block A tail: proj/LN2/FFN/LN1' + gather0.
                        # LN1_A's DVE chain is hidden under KV1-K matmuls
                        # (gather half 1 has arrived by now in steady state).
                        proj_block(0, x_mid)
                        xh2a = ap.tile([P, FT, P], BF16, tag="xh2", bufs=2)
                        ln_block(p_ln2, x_mid, 0, xh2a, oc0=0)
                        ffn_block(0, x_mid, xh2a, x_fin)
                        p_ln1n = ln_params(2 * l + 2)

                        def mid_a():
                            if attn:
                                tc.no_sync_barrier()
                                kv_k(wqkv_t, xf, kf, 1)

                        ln_block(p_ln1n, x_fin, 0, xh_n, mid=mid_a)
                        if not last:
                            launch_gather(xh_n, 0)
                        if attn:
                            kv_v(wqkv_t, xf, 1)
                            attn_block(1)

                        # wqkv(l+1) prefetch: emitted after KV1 so the bufs=1
                        # slot-release (KV1's last wqkv read) precedes it in
                        # scheduler order (avoids a barrier-induced cycle)
                        if not last:
                            wqkv_n = load_wqkv(l + 1)

                        # ---- block B tail ----
                        proj_block(P, x_mid)
                        xh2b = ap.tile([P, FT, P], BF16, tag="xh2", bufs=2)
                        ln_block(p_ln2, x_mid, P, xh2b, oc0=0)
                        ffn_block(P, x_mid, xh2b, x_fin)
                        if not last:
                            # xf half-0 loads for layer l+1: emitted before
                            # the stage-B DMA so the in-order SP queue can't
                            # head-of-line block them behind LN1_B'
                            xf_next = alloc_xf()
                            emit_xf_loads(xf_next, 0)
                        ln_block(p_ln1n, x_fin, P, xh_n)
                        if not last:
                            launch_gather(xh_n, 1)
                            emit_xf_loads(xf_next, 1)
                            xf_cur = xf_next
                            wqkv_t = wqkv_n

                        x_cur = x_fin
                        xh = xh_n

                    xhf = xh  # final LN output (index 2L) built in last iter

            # ---- LM head (token-parallel over own 256 tokens) ----
            if head:
                with (
                    tc.tile_pool(name="hpsum", bufs=1, space="PSUM") as hpp,
                    tc.tile_pool(name="head", bufs=1) as hp_pool,
                ):
                    for vb0 in range(0, V, VC2):
                        vw_c = min(VC2, V - vb0)  # last chunk is 256 wide
                        wh_t = hp_pool.tile([P, FT, VC2], BF16, tag="wh",
                                            bufs=3)
                        for j0 in range(0, vw_c, 512):
                            j1 = min(j0 + 512, vw_c)
                            nc.scalar.dma_start(
                                wh_t[:, :, j0:j1],
                                wh_d.ap()[:, :, vb0 + j0 : vb0 + j1]
                                .rearrange("f p m -> p f m"),
                            )
                        for tt in range(2):
                            ps = hpp.tile([P, VC2], F32, tag="h", bufs=4)
                            for v0 in range(0, vw_c, 512):
                                vw = min(512, vw_c - v0)
                                for f in range(FT):
                                    nc.tensor.matmul(
                                        ps[:, v0 : v0 + vw],
                                        xhf[:, f, tt * P : (tt + 1) * P],
                                        wh_t[:, f, v0 : v0 + vw],
                                        start=(f == 0), stop=(f == FT - 1),
                                    )
                            ob = hp_pool.tile([P, VC2], BF16, tag="ob", bufs=4)
                            nc.scalar.copy(ob[:, 0:vw_c], ps[:, 0:vw_c])
                            nc.sync.dma_start(
                                out_d.ap()[tt, :, vb0 : vb0 + vw_c],
                                ob[:, 0:vw_c],
                            )

    nc.compile()
    return nc


def prep_inputs(inputs):
    """Host-side sharding: returns in_maps (one dict per core)."""
    bf = ml_dtypes.bfloat16
    g = {k: np.asarray(v) for k, v in inputs.items()}
    idx = g["idx"].astype(np.int64)
    tok = np.asarray(g["tok_emb"], np.float32)
    pos = np.asarray(g["pos_emb"], np.float32)

    def fm(w):  # [C_in, M] -> [FT, P, M] bf16
        return np.ascontiguousarray(w.reshape(FT, P, -1)).astype(bf)

    wqkv = np.empty((L, FT, P, 3 * C), bf)
    wp_a = np.empty((L, FT, P, C), bf)
    w1_a = np.empty((L, FT, P, 4 * C), bf)
    w2_a = np.empty((L, F4, P, C), bf)
    for l in range(L):
        q = np.transpose(np.asarray(g["Wq"][l], np.float32), (1, 0, 2)).reshape(C, C)
        k = np.transpose(np.asarray(g["Wk"][l], np.float32), (1, 0, 2)).reshape(C, C)
        v = np.transpose(np.asarray(g["Wv"][l], np.float32), (1, 0, 2)).reshape(C, C)
        wqkv[l] = fm(np.concatenate([q * SCALE, k, v], axis=1))
        wp_a[l] = fm(np.asarray(g["Wp"][l], np.float32))
        w1_a[l] = fm(np.asarray(g["W1"][l], np.float32))
        w2_a[l] = np.asarray(g["W2"][l], np.float32).reshape(F4, P, C).astype(bf)

    lng = np.stack(
        [np.asarray(g["ln1g"][l // 2] if l % 2 == 0 else g["ln2g"][l // 2],
                    np.float32)
         for l in range(2 * L)] + [np.asarray(g["lnfg"], np.float32)]
    ).astype(bf)
    lnb = np.stack(
        [np.asarray(g["ln1b"][l // 2] if l % 2 == 0 else g["ln2b"][l // 2],
                    np.float32)
         for l in range(2 * L)] + [np.asarray(g["lnfb"], np.float32)]
    )

    wh_full = np.asarray(g["Wh"], np.float32).reshape(FT, P, V).astype(bf)

    # per-rank causal masks in S^T ([key, query]) layout, kv blocks in global
    # order: early query block uses kv blocks 0..3, late uses 0..7.
    tri = (np.arange(P)[:, None] <= np.arange(P)[None, :]).astype(np.float32)

    in_maps = []
    for r in range(NCORES):
        bt = r // 4
        lr = r % 4
        blocks = _blocks_of(r)
        e = np.concatenate(
            [tok[idx[bt, gb * P : (gb + 1) * P]] + pos[gb * P : (gb + 1) * P]
             for gb in blocks], axis=0)  # [TB, C]
        x0 = np.ascontiguousarray(
            e.T.reshape(FT, P, TB).transpose(1, 0, 2)).astype(bf)

        m = np.zeros((P, 8 * P), np.float32)
        for ql, gq in enumerate(blocks):
            kbs = range(0, 4) if ql == 0 else range(4, 8)
            for j, kb in enumerate(kbs):
                blk = m[:, (0 if ql == 0 else 4 * P) + j * P :][:, :P]
                if kb < gq:
                    blk[:] = 1.0
                elif kb == gq:
                    blk[:] = tri

        in_maps.append({
            "x0": x0,
            "wqkv": wqkv, "wp": wp_a, "w1": w1_a, "w2": w2_a,
            "wh": wh_full,
            "lng": lng, "lnb": lnb,
            "bp": np.asarray(g["bp"], np.float32),
            "b1": np.asarray(g["b1"], np.float32),
            "b2": np.asarray(g["b2"], np.float32),
            "msk": m.astype(bf),
        })
    return in_maps


_CACHED_NC = None


def kernel(**inputs):
    global _CACHED_NC
    if _CACHED_NC is None:
        _CACHED_NC = build()
    nc = _CACHED_NC
    in_maps = prep_inputs(inputs)
    res = run_bass_kernel_spmd(nc, in_maps, core_ids=list(range(NCORES)))
    logits = np.empty((B, T, V), np.float32)
    for r in range(NCORES):
        bt = r // 4
        out = np.asarray(res.results[r]["out"], np.float32)
        for i, gb in enumerate(_blocks_of(r)):
            logits[bt, gb * P : (gb + 1) * P, :] = out[i]
    return logits


# revision 51
# speedup vs baseline: 1.0061x; 1.0061x over previous
"""GPT forward pass (B=2,T=1024,C=768,H=12,L=6,V=32000) on 8 TRN2 NeuronCores.

Sharding: context/token parallel. Token blocks of 128; batch bt=r//4, local
rank lr=r%4; core r owns query blocks {lr, 7-lr} of its batch (balanced causal
work). Per layer the LN1 activations x-hat (bf16) are all-gathered within each
4-core batch group as TWO half-column collectives (first the early global
blocks 0..3, then 4..7). K/V for the full sequence are recomputed locally from
the gathered x-hat (PE matmul cost is output-columns only, so recompute beats
shipping K/V). LM head is token-parallel (each core: own 256 tokens x full
vocab) so no final collective is needed.

Software pipelining: the per-layer tail (proj/LN2/FFN/LN1') is split per
owned token block so that block A's next-layer LN1 completes mid-layer and
launches gather-half-0 of layer l+1 while block B is still computing; KV0 /
attention / FFN work of layer l then overlaps both in-flight collectives.
Steady state hides nearly all of the ~35us-per-collective latency.

Attention computes scores transposed, S^T[k,q] = K Q^T, over a rank-uniform
fixed set of kv blocks (4 for the early query block, 8 for the late one);
causality and the rank-varying diagonal live in a host-supplied 0/1 mask
multiplied into exp(S^T). No row-max is needed (scores are O(0.1)); the
softmax denominator falls out of a ones-column appended to V, and
normalization is a rank-1 broadcast matmul. No transposes anywhere.

Activations stay feature-major [C_part, token_free]; LN stats/broadcasts via
rank-1 bf16 PE matmuls.
"""

import sys

for _p in (
    "/opt/trn_rl_repo",
    "/opt/pypackages",
    "/root/.axon_site",
    "/root/.axon_site/_ro/trn_rl_repo",
    "/root/.axon_site/_ro/pypackages",
):
    if _p not in sys.path:
        sys.path.append(_p)

import numpy as np
import ml_dtypes

import concourse.bass as bass
import concourse.mybir as mybir
import concourse.tile as tile
from concourse import bacc
from concourse.bass_utils import run_bass_kernel_spmd

BF16 = mybir.dt.bfloat16
F32 = mybir.dt.float32
AF = mybir.ActivationFunctionType
OP = mybir.AluOpType

B, T, C, H, L, V = 2, 1024, 768, 12, 6, 32000
HS, P = 64, 128
NCORES = 8
FT = C // P  # 6 feature tiles
F4 = 4 * C // P  # 24 ffn tiles
TB = 256  # tokens per core
NB = T // P  # 8 blocks per batch sequence
VC2 = 1024  # vocab chunk for the head
EPS = 1e-5
SCALE = C ** -0.5
QW = (4, 8)  # rank-uniform kv-block widths for the two owned query blocks
# global block gb -> column offset in shard-ordered full-sequence buffers
# (shard s contributes its blocks s and 7-s at column s*TB and s*TB+P)
COL = [0, 256, 512, 768, 896, 640, 384, 128]
SHX2 = FT * P * P  # 98304 bf16 elements per half-shard


def _blocks_of(rank):
    lr = rank % 4
    return [lr, 7 - lr]


def build(n_layers=L, attn=True, head=True):
    nc = bacc.Bacc("TRN2", target_bir_lowering=False, debug=False,
                   num_devices=NCORES)

    x0_d = nc.dram_tensor("x0", [P, FT, TB], BF16, kind="ExternalInput")
    wqkv_d = nc.dram_tensor("wqkv", [L, FT, P, 3 * C], BF16, kind="ExternalInput")
    wp_d = nc.dram_tensor("wp", [L, FT, P, C], BF16, kind="ExternalInput")
    w1_d = nc.dram_tensor("w1", [L, FT, P, 4 * C], BF16, kind="ExternalInput")
    w2_d = nc.dram_tensor("w2", [L, F4, P, C], BF16, kind="ExternalInput")
    wh_d = nc.dram_tensor("wh", [FT, P, V], BF16, kind="ExternalInput")
    lng_d = nc.dram_tensor("lng", [2 * L + 1, C], BF16, kind="ExternalInput")
    lnb_d = nc.dram_tensor("lnb", [2 * L + 1, C], F32, kind="ExternalInput")
    bp_d = nc.dram_tensor("bp", [L, C], F32, kind="ExternalInput")
    b1_d = nc.dram_tensor("b1", [L, 4 * C], F32, kind="ExternalInput")
    b2_d = nc.dram_tensor("b2", [L, C], F32, kind="ExternalInput")
    msk_d = nc.dram_tensor("msk", [P, 8 * P], BF16, kind="ExternalInput")
    out_d = nc.dram_tensor("out", [2, P, V], BF16, kind="ExternalOutput")

    with tile.TileContext(nc) as tc:
        with (
            tc.tile_pool(name="const", bufs=1) as cp,
            tc.tile_pool(name="act", bufs=1) as ap,
            tc.tile_pool(name="rows", bufs=2) as rp,
            tc.tile_pool(name="dram", bufs=1, space="DRAM") as dp,
        ):
            # ---- constants ----
            ones_col_bf = cp.tile([P, 1], BF16, name="ones_col_bf")
            nc.vector.memset(ones_col_bf[:], 1.0)
            ones_row_bf = cp.tile([1, P], BF16, name="ones_row_bf")
            nc.vector.memset(ones_row_bf[:], 1.0)
            eps_c = cp.tile([1, 1], F32, name="eps_c")
            nc.vector.memset(eps_c[:], EPS)
            msk = cp.tile([P, 8 * P], BF16, name="msk")
            nc.scalar.dma_start(msk[:], msk_d.ap())

            with tc.tile_pool(name="psum", bufs=1, space="PSUM") as pp:

                def ln_params(i):
                    g_row = rp.tile([1, C], BF16, tag="grow")
                    nc.scalar.dma_start(g_row[:], lng_d.ap()[i : i + 1, :])
                    b_f32 = rp.tile([1, C], F32, tag="bf32")
                    nc.scalar.dma_start(b_f32[:], lnb_d.ap()[i : i + 1, :])
                    b_row = rp.tile([1, C], BF16, tag="brow")
                    nc.vector.tensor_copy(b_row[:], b_f32[:])
                    return g_row, b_row

                def ln_block(params, x_src, c0, out, w=P, oc0=None, mid=None):
                    """LN of x_src[:, :, c0:c0+w] -> out[:, :, oc0:oc0+w].
                    Apply processes f-tiles in pairs (one DVE op covers two)
                    with the bias folded into the ga2 broadcast by PE.
                    `mid` emits independent PE work between the stats chain
                    and the apply so the chain's DVE latency is hidden."""
                    if oc0 is None:
                        oc0 = c0
                    g_row, b_row = params
                    st1 = pp.tile([1, TB], F32, tag="o", bufs=3)
                    st2 = pp.tile([1, TB], F32, tag="o", bufs=3)
                    sqs = []
                    for f in range(FT):
                        sq = ap.tile([P, w], BF16, tag="sq", bufs=3,
                                     name="sq")
                        nc.vector.tensor_tensor(sq[:],
                                                x_src[:, f, c0 : c0 + w],
                                                x_src[:, f, c0 : c0 + w],
                                                op=OP.mult)
                        sqs.append(sq)
                        # all st1 matmuls first: DVE produces the squares
                        # while PE runs these, so st2 rarely waits
                        nc.tensor.matmul(st1[:1, 0:w], ones_col_bf[:],
                                         x_src[:, f, c0 : c0 + w],
                                         start=(f == 0), stop=(f == FT - 1))
                    for f in range(FT):
                        nc.tensor.matmul(st2[:1, 0:w], ones_col_bf[:],
                                         sqs[f][:],
                                         start=(f == 0), stop=(f == FT - 1))
                    rinv, mean, m2, var = (
                        rp.tile([1, w], F32, tag=t, name=t)[:1, :]
                        for t in ("rinv", "mean", "m2", "var")
                    )
                    nc.vector.tensor_scalar_mul(mean, st1[:1, 0:w], 1.0 / C)
                    nc.vector.tensor_tensor(m2, mean, mean, op=OP.mult)
                    nc.vector.scalar_tensor_tensor(
                        var, in0=st2[:1, 0:w], scalar=1.0 / C, in1=m2,
                        op0=OP.mult, op1=OP.subtract,
                    )
                    std = rp.tile([1, w], F32, tag="std", name="std")[:1, :]
                    nc.scalar.activation(std, var, AF.Sqrt, bias=eps_c[:1, :1])
                    rinv_b = rp.tile([1, w], BF16, tag="rinvb", name="rinvb")
                    nmr_b = rp.tile([1, w], BF16, tag="nmrb", name="nmrb")
                    with nc.allow_low_precision(reason="ln scale rows bf16, "
                                                "same as the former f32+copy"):
                        nc.vector.reciprocal(rinv, std)
                        nc.vector.tensor_copy(rinv_b[:1, :], rinv)
                        nc.vector.scalar_tensor_tensor(
                            nmr_b[:1, :], in0=mean, scalar=-1.0, in1=rinv,
                            op0=OP.mult, op1=OP.mult,
                        )
                    if mid is not None:
                        mid()
                    for fp in range(FT // 2):
                        f0 = 2 * fp
                        ga = pp.tile([P, 4 * P], F32, tag="g", bufs=3)
                        for j in range(2):
                            fj = f0 + j
                            nc.tensor.matmul(
                                ga[:, j * w : j * w + w],
                                g_row[:1, fj * P : (fj + 1) * P],
                                rinv_b[:1, :], start=True, stop=True)
                            nc.tensor.matmul(
                                ga[:, (2 + j) * w : (2 + j) * w + w],
                                g_row[:1, fj * P : (fj + 1) * P],
                                nmr_b[:1, :], start=True, stop=False)
                            nc.tensor.matmul(
                                ga[:, (2 + j) * w : (2 + j) * w + w],
                                b_row[:1, fj * P : (fj + 1) * P],
                                ones_row_bf[:1, 0:w], start=False, stop=True)
                        t1 = ap.tile([P, 2, w], BF16, tag="t1", bufs=2)
                        nc.vector.tensor_tensor(
                            t1[:], x_src[:, f0 : f0 + 2, c0 : c0 + w],
                            ga[:, 0 : 2 * w], op=OP.mult)
                        nc.vector.tensor_tensor(
                            out[:, f0 : f0 + 2, oc0 : oc0 + w], t1[:],
                            ga[:, 2 * w : 4 * w], op=OP.add)

                # ---- embedding: host supplies feature-major bf16 tok+pos ----
                x_cur = ap.tile([P, FT, TB], BF16, tag="x", bufs=2)
                nc.sync.dma_start(x_cur[:], x0_d.ap())

                # ---- internal DRAM for the split x-hat all-gather ----
                ag_in = [dp.tile([1, SHX2], BF16, name=f"ag_in{i}")
                         for i in range(2)]
                ag_out = [dp.tile([4, SHX2], BF16, name=f"ag_out{i}")
                          for i in range(2)]

                def launch_gather(xh_t, hf):
                    """Stage xh block hf (local cols hf*P..) and all-gather.
                    Staged partition-major so the xf reload gets >=512B
                    contiguous elements (no 2x DMA latency penalty)."""
                    nc.sync.dma_start(
                        ag_in[hf][0].rearrange("(p f t) -> p f t", f=FT, t=P),
                        xh_t[:, :, hf * P : (hf + 1) * P],
                    )
                    nc.gpsimd.collective_compute(
                        "AllGather", OP.bypass,
                        replica_groups=[[0, 1, 2, 3], [4, 5, 6, 7]],
                        ins=[ag_in[hf][:].opt()],
                        outs=[ag_out[hf][:].opt()],
                    )

                # persistent V (token-major; 65th column stays 1.0 so the
                # softmax denominator falls out of the PV matmul)
                v_sb = ap.tile([P, NB, H, HS + 1], BF16, tag="vsb", bufs=1)
                nc.vector.memset(v_sb[:], 1.0)

                def alloc_xf():
                    # xf laid out [p, half, shard, f, t] so each shard load is
                    # one DMA with 1536B contiguous elements
                    return ap.tile([P, 2, 4, FT, P], BF16, tag="xf",
                                   name="xf")

                def alloc_kf():
                    return ap.tile([P, FT, T], BF16, tag="kf", name="kf")

                def kv_k(wq_t, xf_t, kf_t, hf):
                    """K for the 4 global blocks of half hf. One PSUM tile
                    per out-tile spanning all 4 shards -> one copy each;
                    copies alternate DVE/Act to halve queue pressure."""
                    for o in range(FT):
                        ps = pp.tile([P, 4 * P], F32, tag="g", bufs=3)
                        for s in range(4):
                            for f in range(FT):
                                nc.tensor.matmul(
                                    ps[:, s * P : (s + 1) * P],
                                    wq_t[:, f, C + o * P : C + (o + 1) * P],
                                    xf_t[:, hf, s, f, :],
                                    start=(f == 0), stop=(f == FT - 1),
                                )
                        # shard s of half hf -> kf cols s*TB + hf*P
                        nc.vector.tensor_copy(
                            kf_t[:, o, :].rearrange(
                                "p (s u r) -> p s u r", u=2, r=P)[:, :, hf, :],
                            ps[:].rearrange("p (s r) -> p s r", r=P))

                def kv_v(wq_t, xf_t, hf):
                    """V for the 4 global blocks of half hf, into v_sb.
                    Two 1-bank PSUM chunks per shard (heads 0-7 / 8-11)."""
                    for s in range(4):
                        vb = (s * TB + hf * P) // P
                        for i, (v0, v1, h0, h1) in enumerate(
                                ((0, 512, 0, 8), (512, C, 8, H))):
                            ps = pp.tile([P, 512], F32, tag="s", bufs=2)
                            for f in range(FT):
                                nc.tensor.matmul(
                                    ps[:, 0 : v1 - v0],
                                    xf_t[:, hf, s, f, :],
                                    wq_t[:, f, 2 * C + v0 : 2 * C + v1],
                                    start=(f == 0), stop=(f == FT - 1),
                                )
                            nc.vector.tensor_copy(
                                v_sb[:, vb, h0:h1, 0:HS],
                                ps[:, 0 : v1 - v0].rearrange(
                                    "p (h s) -> p h s", s=HS))

                def emit_xf_loads(xf_t, hf):
                    """SP-queue loads of gathered x-hat; emitted right after
                    the corresponding gather launch so they sit ahead of the
                    next staging DMA in the in-order SP queue."""
                    for s in range(4):
                        nc.sync.dma_start(
                            xf_t[:, hf, s, :, :],
                            ag_out[hf][s].rearrange(
                                "(p f t) -> p f t", f=FT, t=P),
                        )

                with tc.tile_pool(name="wts", bufs=1) as wp_pool:

                    def load_wqkv(l):
                        # bufs=1: the reload window (KV1(l) done -> Q(l+1)) is
                        # ~35us vs a ~10us DMA, so single-buffering is free
                        wq_t = wp_pool.tile([P, FT, 3 * C], BF16,
                                            tag="wqkv", bufs=1)
                        for j in range(3):
                            nc.scalar.dma_start(
                                wq_t[:, :, j * C : (j + 1) * C],
                                wqkv_d.ap()[l, :, :, j * C : (j + 1) * C]
                                .rearrange("f p m -> p f m"))
                        return wq_t

                    # ---- prelude: LN1(layer 0) + both gathers ----
                    wqkv_t = load_wqkv(0)
                    xh = ap.tile([P, FT, TB], BF16, tag="xh", bufs=2)
                    p_ln1 = ln_params(0)
                    ln_block(p_ln1, x_cur, 0, xh)
                    launch_gather(xh, 0)
                    ln_block(p_ln1, x_cur, P, xh)
                    launch_gather(xh, 1)
                    xf_cur = alloc_xf()
                    emit_xf_loads(xf_cur, 0)
                    emit_xf_loads(xf_cur, 1)

                    for l in range(n_layers):
                        # -- bias rows for this layer (Act queue) --
                        bpc = rp.tile([P, FT], F32, tag="bpc")
                        nc.scalar.dma_start(
                            bpc[:], bp_d.ap()[l].rearrange("(f p) -> p f", p=P))
                        b1c = rp.tile([P, F4], F32, tag="b1c")
                        nc.scalar.dma_start(
                            b1c[:], b1_d.ap()[l].rearrange("(f p) -> p f", p=P))
                        b2c = rp.tile([P, FT], F32, tag="b2c")
                        nc.scalar.dma_start(
                            b2c[:], b2_d.ap()[l].rearrange("(f p) -> p f", p=P))

                        # -- Q for own tokens (overlaps in-flight gathers) --
                        q_sb = ap.tile([P, FT, TB], BF16, tag="q")
                        for o in range(FT):
                            ps = pp.tile([P, TB], F32, tag="g", bufs=3)
                            for f in range(FT):
                                nc.tensor.matmul(
                                    ps[:], wqkv_t[:, f, o * P : (o + 1) * P],
                                    xh[:, f, :], start=(f == 0),
                                    stop=(f == FT - 1),
                                )
                            nc.scalar.copy(q_sb[:, o, :], ps[:])

                        # -- this layer's remaining weights (Act HWDGE) --
                        wp_t = wp_pool.tile([P, FT, C], BF16, tag="wp")
                        nc.scalar.dma_start(
                            wp_t[:], wp_d.ap()[l].rearrange("f p m -> p f m"))
                        w1c = []
                        for ch in range(2):
                            w1ct = wp_pool.tile([P, FT, 2 * C], BF16, tag="w1",
                                                bufs=2, name=f"w1c{ch}")
                            w1c.append(w1ct)
                            for j in range(2):
                                m0 = ch * 2 * C + j * C
                                nc.scalar.dma_start(
                                    w1ct[:, :, j * C : (j + 1) * C],
                                    w1_d.ap()[l, :, :, m0 : m0 + C]
                                    .rearrange("f p m -> p f m"))
                        w2c = []
                        for ch in range(2):
                            w2ct = wp_pool.tile([P, F4, 3 * P], BF16,
                                                tag="w2", bufs=2,
                                                name=f"w2c{ch}")
                            w2c.append(w2ct)
                            for j in range(2):
                                nc.scalar.dma_start(
                                    w2ct[:, j * 12 : (j + 1) * 12, :],
                                    w2_d.ap()[l, j * 12 : (j + 1) * 12, :,
                                              ch * 3 * P : (ch + 1) * 3 * P]
                                    .rearrange("f p m -> p f m"))

                        kf = alloc_kf()
                        xf = xf_cur
                        o_fm = ap.tile([P, FT, TB], BF16, tag="ofm")

                        def attn_block(ql):
                            """Attention for owned query block ql (0=early).
                            3-stage software pipeline over heads so PE always
                            has the next head's scores while the softmax
                            chains (exp/mask, recip) of earlier heads drain.
                            Scores/softmax chunked by 4 kv blocks so score
                            PSUM tiles stay one bank."""
                            W = QW[ql]

                            def stage_scores(h):
                                hp, f = HS * (h % 2), h // 2
                                pts = []
                                for c in range(W // 4):
                                    s_ps = pp.tile([P, 4 * P], F32, tag="s",
                                                   bufs=2)
                                    for j in range(4):
                                        kb = 4 * c + j
                                        nc.tensor.matmul(
                                            s_ps[:, j * P : (j + 1) * P],
                                            kf[hp : hp + HS, f,
                                               COL[kb] : COL[kb] + P],
                                            q_sb[hp : hp + HS, f,
                                                 ql * P : (ql + 1) * P],
                                            start=True, stop=True,
                                        )
                                    pt = ap.tile([P, 4 * P], BF16,
                                                 tag="pt", bufs=4, name="pt")
                                    nc.scalar.activation(pt[:], s_ps[:],
                                                         AF.Exp)
                                    # kv blocks 0..3 under a late query block
                                    # (>=4) are fully causal: no mask needed
                                    if ql == 0 or c == 1:
                                        nc.vector.tensor_tensor(
                                            pt[:], pt[:],
                                            msk[:, (0 if ql == 0 else 4 * P):]
                                            [:, : 4 * P], op=OP.mult)
                                    pts.append(pt)
                                return pts

                            def stage_av(h, pts):
                                ov = pp.tile([P, TB], F32, tag="o", bufs=3)
                                for c, pt in enumerate(pts):
                                    for j in range(4):
                                        kb = 4 * c + j
                                        nc.tensor.matmul(
                                            ov[: HS + 1, 0:P],
                                            v_sb[:, COL[kb] // P, h, :],
                                            pt[:, j * P : (j + 1) * P],
                                            start=(kb == 0),
                                            stop=(kb == W - 1),
                                        )
                                rden = rp.tile([1, P], BF16, tag="rden",
                                               bufs=2)
                                with nc.allow_low_precision(
                                        reason="softmax rden bf16"):
                                    nc.vector.reciprocal(rden[:1, :],
                                                         ov[HS : HS + 1, 0:P])
                                return ov, rden

                            def stage_fin(h, ov, rden):
                                hp, f = HS * (h % 2), h // 2
                                nc.tensor.matmul(
                                    ov[0:HS, P : P + P],
                                    ones_row_bf[:1, 0:HS], rden[:1, :],
                                    start=True, stop=True,
                                )
                                sc_sb = ap.tile([HS, P], BF16, tag="scsb",
                                                bufs=3)
                                nc.scalar.copy(sc_sb[:], ov[0:HS, P : P + P])
                                nc.vector.tensor_tensor(
                                    o_fm[hp : hp + HS, f,
                                         ql * P : (ql + 1) * P],
                                    ov[0:HS, 0:P], sc_sb[:],
                                    op=OP.mult,
                                )

                            pts = [None] * H
                            avs = [None] * H
                            for h in range(H):
                                pts[h] = stage_scores(h)
                                if h >= 1:
                                    avs[h - 1] = stage_av(h - 1, pts[h - 1])
                                if h >= 2:
                                    stage_fin(h - 2, *avs[h - 2])
                            avs[H - 1] = stage_av(H - 1, pts[H - 1])
                            stage_fin(H - 2, *avs[H - 2])
                            stage_fin(H - 1, *avs[H - 1])

                        def proj_block(c0, x_new):
                            """Output projection + residual for token block."""
                            for o in range(FT):
                                ps = pp.tile([P, TB], F32, tag="g", bufs=3)
                                for f in range(FT):
                                    nc.tensor.matmul(
                                        ps[:, 0:P],
                                        wp_t[:, f, o * P : (o + 1) * P],
                                        o_fm[:, f, c0 : c0 + P],
                                        start=(f == 0), stop=(f == FT - 1),
                                    )
                                nc.vector.scalar_tensor_tensor(
                                    x_new[:, o, c0 : c0 + P], in0=ps[:, 0:P],
                                    scalar=bpc[:, o : o + 1],
                                    in1=x_cur[:, o, c0 : c0 + P],
                                    op0=OP.add, op1=OP.add,
                                )

                        def ffn_block(c0, x_mid, xh2, x_fin):
                            """FFN + residual for token block at c0. xh2 is
                            block-local [P, FT, P]."""
                            h1 = ap.tile([P, F4, P], BF16, tag="h1", bufs=1)
                            for o in range(F4):
                                ps = pp.tile([P, TB], F32, tag="g", bufs=3)
                                for f in range(FT):
                                    nc.tensor.matmul(
                                        ps[:, 0:P],
                                        w1c[o // 12][:, f,
                                                     (o % 12) * P : (o % 12 + 1) * P],
                                        xh2[:, f, :],
                                        start=(f == 0), stop=(f == FT - 1),
                                    )
                                nc.scalar.activation(h1[:, o, :], ps[:, 0:P],
                                                     AF.Relu,
                                                     bias=b1c[:, o : o + 1])
                            for o in range(FT):
                                ps = pp.tile([P, TB], F32, tag="g", bufs=3)
                                for f in range(F4):
                                    nc.tensor.matmul(
                                        ps[:, 0:P],
                                        w2c[o // 3][:, f,
                                                    (o % 3) * P : (o % 3 + 1) * P],
                                        h1[:, f, :],
                                        start=(f == 0), stop=(f == F4 - 1),
                                    )
                                nc.vector.scalar_tensor_tensor(
                                    x_fin[:, o, c0 : c0 + P], in0=ps[:, 0:P],
                                    scalar=b2c[:, o : o + 1],
                                    in1=x_mid[:, o, c0 : c0 + P],
                                    op0=OP.add, op1=OP.add,
                                )

                        last = l == n_layers - 1

                        x_mid = ap.tile([P, FT, TB], BF16, tag="xm", bufs=2)
                        x_fin = ap.tile([P, FT, TB], BF16, tag="x", bufs=2)
                        xh_n = ap.tile([P, FT, TB], BF16, tag="xh", bufs=2)
                        p_ln2 = ln_params(2 * l + 1)

                        if attn:
                            # scheduler-only fence: keep the collective-gated
                            # KV work from being hoisted ahead of ready work
                            tc.no_sync_barrier()
                            kv_k(wqkv_t, xf, kf, 0)
                            kv_v(wqkv_t, xf, 0)
                            attn_block(0)
                        else:
                            for f in range(FT):
                                nc.vector.tensor_copy(o_fm[:, f, :],
                                                      xh[:, f, :])

                        # ---- block A tail: proj/LN2/FFN/LN1' + gather0.
                        # LN1_A's DVE chain is hidden under KV1-K matmuls
                        # (gather half 1 has arrived by now in steady state).
                        proj_block(0, x_mid)
                        xh2a = ap.tile([P, FT, P], BF16, tag="xh2", bufs=2)
                        ln_block(p_ln2, x_mid, 0, xh2a, oc0=0)
                        ffn_block(0, x_mid, xh2a, x_fin)
                        p_ln1n = ln_params(2 * l + 2)

                        def mid_a():
                            if attn:
                                tc.no_sync_barrier()
                                kv_k(wqkv_t, xf, kf, 1)

                        ln_block(p_ln1n, x_fin, 0, xh_n, mid=mid_a)
                        if not last:
                            launch_gather(xh_n, 0)
                        if attn:
                            kv_v(wqkv_t, xf, 1)
                            attn_block(1)

                        # wqkv(l+1) prefetch: emitted after KV1 so the bufs=1
                        # slot-release (KV1's last wqkv read) precedes it in
                        # scheduler order (avoids a barrier-induced cycle)
                        if not last:
                            wqkv_n = load_wqkv(l + 1)

                        # ---- block B tail ----
                        proj_block(P, x_mid)
                        xh2b = ap.tile([P, FT, P], BF16, tag="xh2", bufs=2)
                        ln_block(p_ln2, x_mid, P, xh2b, oc0=0)
                        ffn_block(P, x_mid, xh2b, x_fin)
                        if not last:
                            # xf half-0 loads for layer l+1: emitted before
                            # the stage-B DMA so the in-order SP queue can't
                            # head-of-line block them behind LN1_B'
                            xf_next = alloc_xf()
                            emit_xf_loads(xf_next, 0)
                        ln_block(p_ln1n, x_fin, P, xh_n)
                        if not last:
                            launch_gather(xh_n, 1)
                            emit_xf_loads(xf_next, 1)
                            xf_cur = xf_next
                            wqkv_t = wqkv_n

                        x_cur = x_fin
                        xh = xh_n

                    xhf = xh  # final LN output (index 2L) built in last iter

            # ---- LM head (token-parallel over own 256 tokens) ----
            if head:
                with (
                    tc.tile_pool(name="hpsum", bufs=1, space="PSUM") as hpp,
                    tc.tile_pool(name="head", bufs=1) as hp_pool,
                ):
                    for vb0 in range(0, V, VC2):
                        vw_c = min(VC2, V - vb0)  # last chunk is 256 wide
                        wh_t = hp_pool.tile([P, FT, VC2], BF16, tag="wh",
                                            bufs=4)
                        for j0 in range(0, vw_c, 512):
                            j1 = min(j0 + 512, vw_c)
                            nc.scalar.dma_start(
                                wh_t[:, :, j0:j1],
                                wh_d.ap()[:, :, vb0 + j0 : vb0 + j1]
                                .rearrange("f p m -> p f m"),
                            )
                        for tt in range(2):
                            ps = hpp.tile([P, VC2], F32, tag="h", bufs=4)
                            for v0 in range(0, vw_c, 512):
                                vw = min(512, vw_c - v0)
                                for f in range(FT):
                                    nc.tensor.matmul(
                                        ps[:, v0 : v0 + vw],
                                        xhf[:, f, tt * P : (tt + 1) * P],
                                        wh_t[:, f, v0 : v0 + vw],
                                        start=(f == 0), stop=(f == FT - 1),
                                    )
                            ob = hp_pool.tile([P, VC2], BF16, tag="ob", bufs=4)
                            nc.scalar.copy(ob[:, 0:vw_c], ps[:, 0:vw_c])
                            nc.sync.dma_start(
                                out_d.ap()[tt, :, vb0 : vb0 + vw_c],
                                ob[:, 0:vw_c],
                            )

    nc.compile()
    return nc


def prep_inputs(inputs):
    """Host-side sharding: returns in_maps (one dict per core)."""
    bf = ml_dtypes.bfloat16
    g = {k: np.asarray(v) for k, v in inputs.items()}
    idx = g["idx"].astype(np.int64)
    tok = np.asarray(g["tok_emb"], np.float32)
    pos = np.asarray(g["pos_emb"], np.float32)

    def fm(w):  # [C_in, M] -> [FT, P, M] bf16
        return np.ascontiguousarray(w.reshape(FT, P, -1)).astype(bf)

    wqkv = np.empty((L, FT, P, 3 * C), bf)
    wp_a = np.empty((L, FT, P, C), bf)
    w1_a = np.empty((L, FT, P, 4 * C), bf)
    w2_a = np.empty((L, F4, P, C), bf)
    for l in range(L):
        q = np.transpose(np.asarray(g["Wq"][l], np.float32), (1, 0, 2)).reshape(C, C)
        k = np.transpose(np.asarray(g["Wk"][l], np.float32), (1, 0, 2)).reshape(C, C)
        v = np.transpose(np.asarray(g["Wv"][l], np.float32), (1, 0, 2)).reshape(C, C)
        wqkv[l] = fm(np.concatenate([q * SCALE, k, v], axis=1))
        wp_a[l] = fm(np.asarray(g["Wp"][l], np.float32))
        w1_a[l] = fm(np.asarray(g["W1"][l], np.float32))
        w2_a[l] = np.asarray(g["W2"][l], np.float32).reshape(F4, P, C).astype(bf)

    lng = np.stack(
        [np.asarray(g["ln1g"][l // 2] if l % 2 == 0 else g["ln2g"][l // 2],
                    np.float32)
         for l in range(2 * L)] + [np.asarray(g["lnfg"], np.float32)]
    ).astype(bf)
    lnb = np.stack(
        [np.asarray(g["ln1b"][l // 2] if l % 2 == 0 else g["ln2b"][l // 2],
                    np.float32)
         for l in range(2 * L)] + [np.asarray(g["lnfb"], np.float32)]
    )

    wh_full = np.asarray(g["Wh"], np.float32).reshape(FT, P, V).astype(bf)

    # per-rank causal masks in S^T ([key, query]) layout, kv blocks in global
    # order: early query block uses kv blocks 0..3, late uses 0..7.
    tri = (np.arange(P)[:, None] <= np.arange(P)[None, :]).astype(np.float32)

    in_maps = []
    for r in range(NCORES):
        bt = r // 4
        lr = r % 4
        blocks = _blocks_of(r)
        e = np.concatenate(
            [tok[idx[bt, gb * P : (gb + 1) * P]] + pos[gb * P : (gb + 1) * P]
             for gb in blocks], axis=0)  # [TB, C]
        x0 = np.ascontiguousarray(
            e.T.reshape(FT, P, TB).transpose(1, 0, 2)).astype(bf)

        m = np.zeros((P, 8 * P), np.float32)
        for ql, gq in enumerate(blocks):
            kbs = range(0, 4) if ql == 0 else range(4, 8)
            for j, kb in enumerate(kbs):
                blk = m[:, (0 if ql == 0 else 4 * P) + j * P :][:, :P]
                if kb < gq:
                    blk[:] = 1.0
                elif kb == gq:
                    blk[:] = tri

        in_maps.append({
            "x0": x0,
            "wqkv": wqkv, "wp": wp_a, "w1": w1_a, "w2": w2_a,
            "wh": wh_full,
            "lng": lng, "lnb": lnb,
            "bp": np.asarray(g["bp"], np.float32),
            "b1": np.asarray(g["b1"], np.float32),
            "b2": np.asarray(g["b2"], np.float32),
            "msk": m.astype(bf),
        })
    return in_maps


_CACHED_NC = None


def kernel(**inputs):
    global _CACHED_NC
    if _CACHED_NC is None:
        _CACHED_NC = build()
    nc = _CACHED_NC
    in_maps = prep_inputs(inputs)
    res = run_bass_kernel_spmd(nc, in_maps, core_ids=list(range(NCORES)))
    logits = np.empty((B, T, V), np.float32)
    for r in range(NCORES):
        bt = r // 4
        out = np.asarray(res.results[r]["out"], np.float32)
        for i, gb in enumerate(_blocks_of(r)):
            logits[bt, gb * P : (gb + 1) * P, :] = out[i]
    return logits
